# revision 1
# baseline (speedup 1.0000x reference)
"""Trainium2 Bass kernel for nn_CrossAttention (dual-modality BN + spatial/channel
cross-attention, B=8, C=128, H=W=128).

Strategy: data-parallel over batch (one sample per NeuronCore, 8 cores).
Two SPMD launches:
  1. stats kernel  — per-core per-channel mean/var of rgb & thermal (bn_stats/bn_aggr).
     Host combines per-core stats into exact global training-mode BN statistics.
  2. main kernel   — all the attention math. BatchNorm, softmax scales, sigmoid
     gates and most biases are folded into the 1x1-conv weights on the host
     (cheap [128,128] manipulations); all heavy compute runs on device in bf16
     matmuls with fp32 PSUM accumulation and an exact fp32 residual add.

Self-contained: only numpy + concourse needed.
"""

from contextlib import ExitStack

import numpy as np

import concourse.mybir as mybir
import concourse.tile as tile
from concourse import bacc
from concourse.bass_utils import run_bass_kernel_spmd
from concourse.masks import make_identity

# Problem dims (hardcoded per spec)
B, C, H, W = 8, 128, 128, 128
NH, P = 4, 8
HD = C // NH            # 32 head dim
HW = H * W              # 16384
NHP = H // P            # 16 patches per side
X = NHP * NHP           # 256 patches
NOFF = P * P            # 64 within-patch offsets
EPS = 1e-5
N_CORES = 8

F32 = mybir.dt.float32
BF16 = mybir.dt.bfloat16
AF = mybir.ActivationFunctionType
AX = mybir.AxisListType

# info about the last run, for test harness introspection
LAST_RUN_INFO = {}

# dev-only phase toggles for timeline attribution (all True in production)
PHASES = {"load": True, "cprep": True, "sa": True, "final": True}


# --------------------------------------------------------------------------
# Stats kernel: per-channel mean/var of both modalities for one sample.
# --------------------------------------------------------------------------
def _emit_stats(tc):
    nc = tc.nc
    xr = nc.dram_tensor("xr", [C, HW], F32, kind="ExternalInput").ap()
    xt = nc.dram_tensor("xt", [C, HW], F32, kind="ExternalInput").ap()
    out = nc.dram_tensor("stats", [C, 4], F32, kind="ExternalOutput").ap()

    with ExitStack() as ctx:
        ld = ctx.enter_context(tc.tile_pool(name="ld", bufs=3))
        acc = ctx.enter_context(tc.tile_pool(name="acc", bufs=1))

        TF = 512  # load tile free size (one bn_stats consumer per DMA)
        NT = HW // TF
        stats_sb = acc.tile([C, 2, NT, 6], F32)
        agg = acc.tile([C, 4], F32)
        for t, xd in ((0, xr), (1, xt)):
            for i in range(NT):
                lt = ld.tile([C, TF], F32, name="lt", tag="lt")
                nc.sync.dma_start(lt[:], xd[:, i * TF:(i + 1) * TF])
                nc.vector.bn_stats(out=stats_sb[:, t, i, :], in_=lt[:])
            nc.vector.bn_aggr(out=agg[:, 2 * t:2 * t + 2], in_=stats_sb[:, t, :, :])
        nc.sync.dma_start(out[:, :], agg[:])


def _build_stats():
    nc = bacc.Bacc("TRN2")
    with tile.TileContext(nc) as tc:
        _emit_stats(tc)
    nc.compile()
    return nc


# --------------------------------------------------------------------------
# Main kernel
# --------------------------------------------------------------------------
def _grid(ap, ph, pw):
    """[C, HW] AP -> [C, NHP, NHP] grid slice at within-patch offset (ph,pw)."""
    v = ap.rearrange("c (a p b q) -> c a p b q", a=NHP, p=P, b=NHP, q=P)
    return v[:, :, ph, :, pw]


def _grid2(ap, ph, pw):
    """[C, HW] AP -> [C, 2, NHP, NHP]: offsets (ph,pw) and (ph,pw+1),
    pair-major so each offset's 256 grid pixels are contiguous in stream
    order."""
    v = ap.rearrange("c (a p b q) -> c a p b q", a=NHP, p=P, b=NHP, q=P)
    return v[:, :, ph, :, pw:pw + 2].rearrange("c a b q -> c q a b")


class _Evict:
    """Alternate PSUM->SBUF evictions between the scalar(ACT) and vector(DVE)
    engines to balance load."""

    def __init__(self, nc):
        self.nc = nc
        self.i = 0

    def __call__(self, out_ap, in_ap, bias=None):
        nc = self.nc
        # ACT copies cost ~2x DVE copies; give ACT every third eviction
        use_act = (self.i % 3) == 0
        self.i += 1
        if bias is None:
            if use_act:
                nc.scalar.copy(out_ap, in_ap)
            else:
                nc.vector.tensor_copy(out_ap, in_ap)
        else:
            if use_act:
                nc.scalar.activation(out_ap, in_ap, AF.Identity, bias=bias)
            else:
                nc.vector.tensor_scalar_add(out_ap, in_ap, bias)


def _emit_main(tc):
    nc = tc.nc

    # ---- DRAM I/O ----
    xr_d = nc.dram_tensor("xr", [C, HW], F32, kind="ExternalInput").ap()
    xt_d = nc.dram_tensor("xt", [C, HW], F32, kind="ExternalInput").ap()

    def win(name, cols=C):
        return nc.dram_tensor(name, [C, cols], BF16, kind="ExternalInput").ap()

    def bin_(name):
        return nc.dram_tensor(name, [C, 1], F32, kind="ExternalInput").ap()

    wd = {}
    for m in ("r", "t"):
        for nm in ("qwT", "kwT", "vwT", "pwT"):
            wd[f"sa_{m}_{nm}"] = win(f"sa_{m}_{nm}")
        wd[f"ca_from_{m}"] = win(f"ca_from_{m}", 2 * C)
        wd[f"ca_{m}_vwT"] = win(f"ca_{m}_vwT")
        wd[f"ca_{m}_pwT"] = win(f"ca_{m}_pwT")
        wd[f"ca_{m}_vb"] = nc.dram_tensor(
            f"ca_{m}_vb", [C, 1], BF16, kind="ExternalInput"
        ).ap()
        wd[f"sa_{m}_qb"] = bin_(f"sa_{m}_qb")
        wd[f"sa_{m}_kb"] = bin_(f"sa_{m}_kb")
        wd[f"pb_comb_{m}"] = bin_(f"pb_comb_{m}")
        wd[f"gcorr_{m}"] = nc.dram_tensor(
            f"gcorr_{m}", [C, HD], F32, kind="ExternalInput"
        ).ap()

    out_d = nc.dram_tensor("out", [2 * C, HW], F32, kind="ExternalOutput").ap()

    with ExitStack() as ctx:
        # ---- pools ----
        res = ctx.enter_context(tc.tile_pool(name="res", bufs=1))
        wpool = ctx.enter_context(tc.tile_pool(name="wpool", bufs=1))
        ldp = ctx.enter_context(tc.tile_pool(name="ldp", bufs=4))
        rp = ctx.enter_context(tc.tile_pool(name="rp", bufs=6))
        sp = ctx.enter_context(tc.tile_pool(name="sp", bufs=4))      # rotating sbuf
        smp = ctx.enter_context(tc.tile_pool(name="smp", bufs=8))    # small [128,1]
        pp_acc = ctx.enter_context(tc.tile_pool(name="pp_acc", bufs=1, space="PSUM"))
        pp_rot = ctx.enter_context(tc.tile_pool(name="pp_rot", bufs=4, space="PSUM"))

        ev = _Evict(nc)

        # ---- load weights ----
        wt = {}
        for k, ap in wd.items():
            t = wpool.tile(list(ap.shape), ap.dtype, tag=k)
            nc.sync.dma_start(t[:], ap)
            wt[k] = t

        ident = wpool.tile([C, C], BF16, name="ident", tag="ident")
        make_identity(nc, ident[:])

        # ---- load inputs, cast to resident bf16 ----
        # interleave the two tensors' slices so C-prep (which needs early
        # slices of BOTH) can start while the tail is still loading
        xb = {}
        for name in ("r", "t"):
            xb[name] = res.tile([C, HW], BF16, name=f"x{name}_bf",
                                tag=f"x{name}_bf")
        TF = 1024
        for i in range(HW // TF):
            for name, xd in (("r", xr_d), ("t", xt_d)):
                lt = ldp.tile([C, TF], F32, name="in_ld", tag="in_ld")
                nc.sync.dma_start(lt[:], xd[:, i * TF:(i + 1) * TF])
                ev(xb[name][:, i * TF:(i + 1) * TF], lt[:])

        # persistent spatial buffers (reused across modalities)
        kbuf = res.tile([C, NOFF * X], BF16, name="kbuf", tag="kbuf")
        stbuf = res.tile([C, NH * 2 * X], BF16, name="stbuf", tag="stbuf")
        accum = res.tile([C, HW], BF16, name="accum", tag="accum")

        # ==================================================================
        # Phase C-prep: channel attention grams -> folded pconv matrices
        # ==================================================================
        gram = {}
        if not PHASES["cprep"]:
            mt_sb = {m: wt[f"ca_{m}_pwT"] for m in ("r", "t")}
            bias_base = {m: wt[f"pb_comb_{m}"] for m in ("r", "t")}
        for m in ("r", "t") if PHASES["cprep"] else ():
            gram[m] = pp_acc.tile([C, C], F32, name=f"gram_{m}", tag=f"qk{0 if m==chr(114) else 1}")
        def cprep_grams(cps, blk):
            first, last = blk == 0, blk == (HW // C) - 1
            # gram_r = q_r^T k_r : q_r in xr-pack cols 0:C, k_r in xt-pack cols C:2C
            nc.tensor.matmul(
                gram["r"][:], lhsT=cps["r"][:, 0:C], rhs=cps["t"][:, C:2 * C],
                start=first, stop=last,
            )
            nc.tensor.matmul(
                gram["t"][:], lhsT=cps["t"][:, 0:C], rhs=cps["r"][:, C:2 * C],
                start=first, stop=last,
            )

        pendc = None
        for blk in range(HW // C) if PHASES["cprep"] else ():
            cps = {}
            for m in ("r", "t"):
                ps = pp_rot.tile([C, 2 * C], F32, name="cprep_ps", tag="ps")
                nc.tensor.matmul(
                    ps[:],
                    lhsT=xb[m][:, blk * C:(blk + 1) * C],
                    rhs=wt[f"ca_from_{m}"][:],
                    start=True, stop=True,
                )
                sb = sp.tile([C, 2 * C], BF16, name="cprep_sb", tag="cprep_sb")
                ev(sb[:], ps[:])
                cps[m] = sb
            # gram matmuls for block N-1 are emitted after block N's convs so
            # the PE never stalls on the evictions
            if pendc is not None:
                cprep_grams(*pendc)
            pendc = (cps, blk)
        if pendc is not None:
            cprep_grams(*pendc)

        # softmax over per-head diagonal blocks + fold pw through
        if PHASES["cprep"]:
            mt_sb = {}
            bias_base = {}
        for m in ("r", "t") if PHASES["cprep"] else ():
            dg = sp.tile([C, HD], F32, name="ca_diag", tag="ca_diag")
            for n in range(NH):
                s = slice(n * HD, (n + 1) * HD)
                nc.vector.tensor_copy(dg[s, :], gram[m][:][s, s])
            nc.vector.tensor_add(dg[:], dg[:], wt[f"gcorr_{m}"][:])
            mx = smp.tile([C, 1], F32, name="mx", tag="mx")
            nc.vector.reduce_max(mx[:], dg[:], axis=AX.X, negate=True)
            ex = sp.tile([C, HD], F32, name="ca_exp", tag="ca_exp")
            nc.scalar.activation(ex[:], dg[:], AF.Exp, bias=mx[:])
            sm = smp.tile([C, 1], F32, name="sm", tag="sm")
            nc.vector.reduce_sum(sm[:], ex[:], axis=AX.X)
            rc = smp.tile([C, 1], F32, name="rc", tag="rc")
            nc.vector.reciprocal(rc[:], sm[:])
            prob = sp.tile([C, HD], BF16, name="ca_prob", tag="ca_prob")
            nc.vector.tensor_scalar_mul(prob[:], ex[:], rc[:])
            # assemble block-diagonal softmax matrix
            bd = sp.tile([C, C], BF16, name="ca_bd", tag="ca_bd")
            nc.vector.memset(bd[:], 0.0)
            for n in range(NH):
                s = slice(n * HD, (n + 1) * HD)
                nc.scalar.copy(bd[:][s, s], prob[s, :])
            # MT = (pw_eff @ S_bd)^T  via  matmul(lhsT=S_bd[i,j], rhs=pwT[i,o])
            mt_ps = pp_rot.tile([C, C], F32, name="mt_ps", tag="ps")
            nc.tensor.matmul(
                mt_ps[:], lhsT=bd[:], rhs=wt[f"ca_{m}_pwT"][:], start=True, stop=True
            )
            mt = wpool.tile([C, C], BF16, name=f"mt_{m}", tag=f"mt_{m}")
            ev(mt[:], mt_ps[:])
            mt_sb[m] = mt
            # bias: M @ vb  (+ pb_comb)
            mvb_ps = pp_rot.tile([C, 1], F32, name="mvb_ps", tag="ps")
            nc.tensor.matmul(
                mvb_ps[:], lhsT=mt[:], rhs=wt[f"ca_{m}_vb"][:], start=True, stop=True
            )
            bb = wpool.tile([C, 1], F32, name=f"bias_base_{m}", tag=f"bias_base_{m}")
            nc.vector.tensor_add(bb[:], mvb_ps[:], wt[f"pb_comb_{m}"][:])
            bias_base[m] = bb

        # ==================================================================
        # Spatial attention + per-modality finalize
        # ==================================================================
        for m, mo in (("r", "t"), ("t", "r")):
            xq, xkv = xb[m], xb[mo]
            w_q, w_k, w_v, w_p = (
                wt[f"sa_{m}_qwT"], wt[f"sa_{m}_kwT"],
                wt[f"sa_{m}_vwT"], wt[f"sa_{m}_pwT"],
            )
            qb, kb = wt[f"sa_{m}_qb"], wt[f"sa_{m}_kb"]

            def conv_qo(ph, pw):
                # conv for the offset pair (ph,pw),(ph,pw+1): [C, 2*X]
                ps = pp_rot.tile([C, 2 * X], F32, name="qo_ps", tag="ps")
                nc.tensor.matmul(
                    ps[:], lhsT=w_q[:], rhs=_grid2(xq[:], ph, pw),
                    start=True, stop=True,
                )
                qo = sp.tile([C, 2 * X], BF16, name="qo", tag="qo")
                ev(qo[:], ps[:], bias=qb[:])
                return qo

            def softmax_transpose(qk_ps, xh):
                # qk_ps: list of 4 PSUM tiles [x_half=128, y=256]; write ST
                for n in range(NH):
                    mx = smp.tile([C, 1], F32, name="mx", tag="mx")
                    nc.vector.reduce_max(mx[:], qk_ps[n][:], axis=AX.X, negate=True)
                    s_sb = sp.tile([C, X], BF16, name="s_sb", tag="s_sb")
                    nc.scalar.activation(s_sb[:], qk_ps[n][:], AF.Exp, bias=mx[:])
                    sm = smp.tile([C, 1], F32, name="sm", tag="sm")
                    nc.vector.reduce_sum(sm[:], s_sb[:], axis=AX.X)
                    rc = smp.tile([C, 1], F32, name="rc", tag="rc")
                    nc.vector.reciprocal(rc[:], sm[:])
                    nc.vector.tensor_scalar_mul(s_sb[:], s_sb[:], rc[:])
                    tp = pp_rot.tile([C, X], BF16, name="tp_ps", tag="ps")
                    nc.tensor.transpose(tp[:, 0:C], s_sb[:, 0:C], ident[:])
                    nc.tensor.transpose(tp[:, C:X], s_sb[:, C:X], ident[:])
                    # ST[yh][:, n-block x-half xh]
                    for yh in range(2):
                        ev(
                            stbuf[:, n * 2 * X + yh * X + xh * C:
                                  n * 2 * X + yh * X + xh * C + C],
                            tp[:, yh * C:(yh + 1) * C],
                        )

            # ---- pass 1: k,v convs + qk accumulation for x-half 0 ----
            if not PHASES["sa"]:
                continue
            def emit_qk(qk_ps, qo, pair, xh):
                # qo holds offsets 2*pair (cols 0:X) and 2*pair+1 (cols X:2X)
                for pp in range(2):
                    off = 2 * pair + pp
                    cs = pp * X + (0 if xh == 0 else C)
                    for n in range(NH):
                        s = slice(n * HD, (n + 1) * HD)
                        nc.tensor.matmul(
                            qk_ps[n][:],
                            lhsT=qo[s, cs:cs + C],
                            rhs=kbuf[s, off * X:(off + 1) * X],
                            tile_position=(n * HD, 0),
                            start=(off == 0), stop=(off == NOFF - 1),
                        )

            NPAIR = NOFF // 2
            # software-pipelined emission: pair p's qk matmuls are emitted
            # after pair p+1's convs so PE never stalls on the evictions
            qk_ps = [pp_acc.tile([C, X], F32, name=f"qk{n}", tag=f"qk{n}") for n in range(NH)]
            pend = []
            for pair in range(NPAIR):
                ph, pw = (2 * pair) // P, (2 * pair) % P
                qo = conv_qo(ph, pw)
                kps = pp_rot.tile([C, 2 * X], F32, name="ko_ps", tag="ps")
                nc.tensor.matmul(
                    kps[:], lhsT=w_k[:], rhs=_grid2(xkv[:], ph, pw),
                    start=True, stop=True,
                )
                ev(kbuf[:, 2 * pair * X:(2 * pair + 2) * X], kps[:], bias=kb[:])
                pend.append((qo, pair))
                if len(pend) > 1:
                    emit_qk(qk_ps, *pend.pop(0), 0)
            for p_ in pend:
                emit_qk(qk_ps, *p_, 0)
            softmax_transpose(qk_ps, 0)

            # ---- pass 2: recompute q convs + qk for x-half 1 ----
            qk_ps = [pp_acc.tile([C, X], F32, name=f"qk{n}", tag=f"qk{n}") for n in range(NH)]
            pend = []
            for pair in range(NPAIR):
                ph, pw = (2 * pair) // P, (2 * pair) % P
                qo = conv_qo(ph, pw)
                pend.append((qo, pair))
                if len(pend) > 1:
                    emit_qk(qk_ps, *pend.pop(0), 1)
            for p_ in pend:
                emit_qk(qk_ps, *p_, 1)
            softmax_transpose(qk_ps, 1)

            # ---- pass 3: qkv (col-tiled) + pconv -> accum (sw-pipelined) ----
            def p3_front(pair):
                ph, pw = (2 * pair) // P, (2 * pair) % P
                # stage the (strided) grid slices contiguously: matmul
                # stationary operands must have a single free dimension
                xg = sp.tile([C, 2 * X], BF16, name="xg_sb", tag="xg_sb")
                ev(xg[:], _grid2(xkv[:], ph, pw))
                vps = pp_rot.tile([C, 2 * X], F32, name="vt_ps", tag="ps")
                for pp in range(2):
                    for h in range(2):
                        cs = pp * X + h * C
                        nc.tensor.matmul(
                            vps[:, cs:cs + C],
                            lhsT=xg[:, cs:cs + C], rhs=w_v[:],
                            start=True, stop=True,
                        )
                vt_sb = sp.tile([C, 2 * X], BF16, name="vt_sb", tag="vt_sb")
                ev(vt_sb[:], vps[:])
                return vt_sb

            def p3_back(vt_sb, pair):
                ph, pw = (2 * pair) // P, (2 * pair) % P
                qkv_ps = pp_rot.tile([C, 2 * X], F32, name="qkv_ps", tag="ps")
                for pp in range(2):
                    for yh in range(2):
                        for n in range(NH):
                            nc.tensor.matmul(
                                qkv_ps[:][n * HD:(n + 1) * HD,
                                          pp * X:(pp + 1) * X],
                                lhsT=vt_sb[:, pp * X + yh * C + n * HD:
                                           pp * X + yh * C + (n + 1) * HD],
                                rhs=stbuf[:, n * 2 * X + yh * X:
                                          n * 2 * X + (yh + 1) * X],
                                tile_position=(0, n * HD),
                                start=(yh == 0), stop=(yh == 1),
                                skip_group_check=True,
                            )
                qkv_sb = sp.tile([C, 2 * X], BF16, name="qkv_sb", tag="qkv_sb")
                ev(qkv_sb[:], qkv_ps[:])
                pc_ps = pp_rot.tile([C, 2 * X], F32, name="pc_ps", tag="ps")
                nc.tensor.matmul(
                    pc_ps[:], lhsT=w_p[:], rhs=qkv_sb[:], start=True, stop=True
                )
                acc_ap = _grid2(accum[:], ph, pw)
                if (pair % 2) == 0:
                    nc.scalar.activation(acc_ap, pc_ps[:], AF.Identity,
                                         bias=bias_base[m][:])
                else:
                    nc.vector.tensor_scalar_add(acc_ap, pc_ps[:], bias_base[m][:])

            pend3 = []
            for pair in range(NPAIR):
                vt_sb = p3_front(pair)
                pend3.append((vt_sb, pair))
                if len(pend3) > 1:
                    p3_back(*pend3.pop(0))
            for p_ in pend3:
                p3_back(*p_)

            # ---- finalize: ca v-conv + fused pconv + residual combine ----
            x_res_d = xr_d if m == "r" else xt_d
            for blk in range(HW // 512) if PHASES["final"] else ():
                sl = slice(blk * 512, (blk + 1) * 512)
                vps = pp_rot.tile([C, 512], F32, name="cav_ps", tag="ps")
                nc.tensor.matmul(
                    vps[:], lhsT=wt[f"ca_{m}_vwT"][:], rhs=xkv[:, sl],
                    start=True, stop=True,
                )
                v_sb = sp.tile([C, 512], BF16, name="cav_sb", tag="cav_sb")
                ev(v_sb[:], vps[:])
                ca_ps = pp_rot.tile([C, 512], F32, name="ca_ps", tag="ps")
                nc.tensor.matmul(
                    ca_ps[:], lhsT=mt_sb[m][:], rhs=v_sb[:], start=True, stop=True
                )
                rt = rp.tile([C, 512], F32, name="resid", tag="resid")
                nc.sync.dma_start(rt[:], x_res_d[:, sl])
                ot = sp.tile([C, 512], F32, name="outt", tag="outt")
                nc.vector.tensor_add(ot[:], ca_ps[:], accum[:, sl])
                nc.gpsimd.tensor_add(ot[:], ot[:], rt[:])
                mi = 0 if m == "r" else 1
                nc.sync.dma_start(out_d[mi * C:(mi + 1) * C, sl], ot[:])


def _build_main():
    nc = bacc.Bacc("TRN2")
    with tile.TileContext(nc) as tc:
        _emit_main(tc)
    nc.compile()
    return nc


# --------------------------------------------------------------------------
# Host-side folding
# --------------------------------------------------------------------------
def _sigmoid(x):
    return 1.0 / (1.0 + np.exp(-np.float64(x)))


def _fold(inputs, core_stats):
    """core_stats: [N_CORES, C, 4] = (mean_r, var_r, mean_t, var_t) per core.
    Returns (replicated_map, per_core_maps)."""
    f8 = np.float64
    means = {"r": core_stats[:, :, 0].astype(f8), "t": core_stats[:, :, 2].astype(f8)}
    var_s = {"r": core_stats[:, :, 1].astype(f8), "t": core_stats[:, :, 3].astype(f8)}
    mu, sg, tsh = {}, {}, {}
    bn_g = {"r": inputs["rgb_bn_g"], "t": inputs["th_bn_g"]}
    bn_b = {"r": inputs["rgb_bn_b"], "t": inputs["th_bn_b"]}
    for m in ("r", "t"):
        mu_m = means[m].mean(axis=0)
        var_m = (var_s[m] + means[m] ** 2).mean(axis=0) - mu_m ** 2
        mu[m] = mu_m
        s = np.asarray(bn_g[m], f8) / np.sqrt(var_m + EPS)
        sg[m] = s
        tsh[m] = np.asarray(bn_b[m], f8) - mu_m * s

    bf = mybir.dt.np(BF16)
    rep = {}
    alpha = {"r": _sigmoid(inputs["rgb_alpha"][0]), "t": _sigmoid(inputs["th_alpha"][0])}
    beta = {"r": _sigmoid(inputs["rgb_beta"][0]), "t": _sigmoid(inputs["th_beta"][0])}
    SC = (HD * P * P) ** -0.5
    CSC = HW ** -0.5

    eff = {}
    for m, mo in (("r", "t"), ("t", "r")):
        pfx = f"sa_{m}"
        qw = np.asarray(inputs[pfx + "_qw"], f8)
        qb = np.asarray(inputs[pfx + "_qb"], f8)
        kvw = np.asarray(inputs[pfx + "_kvw"], f8)
        kvb = np.asarray(inputs[pfx + "_kvb"], f8)
        pw = np.asarray(inputs[pfx + "_pw"], f8)
        pb = np.asarray(inputs[pfx + "_pb"], f8)
        kw, vw = kvw[:C], kvw[C:]
        kb_, vb_ = kvb[:C], kvb[C:]
        qw_e = SC * qw * sg[m][None, :]
        qb_e = SC * (qb + qw @ tsh[m])
        kw_e = kw * sg[mo][None, :]
        kb_e = kb_ + kw @ tsh[mo]
        vw_e = vw * sg[mo][None, :]
        vb_e = vb_ + vw @ tsh[mo]
        pw_e = alpha[m] * pw
        pb_sa = alpha[m] * (pb + pw @ vb_e)
        rep[f"sa_{m}_qwT"] = qw_e.T.astype(bf)
        rep[f"sa_{m}_kwT"] = kw_e.T.astype(bf)
        rep[f"sa_{m}_vwT"] = vw_e.T.astype(bf)
        rep[f"sa_{m}_pwT"] = pw_e.T.astype(bf)
        rep[f"sa_{m}_qb"] = qb_e.reshape(C, 1).astype(np.float32)
        rep[f"sa_{m}_kb"] = kb_e.reshape(C, 1).astype(np.float32)

        pfx = f"ca_{m}"
        cqw = np.asarray(inputs[pfx + "_qw"], f8)
        cqb = np.asarray(inputs[pfx + "_qb"], f8)
        ckvw = np.asarray(inputs[pfx + "_kvw"], f8)
        ckvb = np.asarray(inputs[pfx + "_kvb"], f8)
        cpw = np.asarray(inputs[pfx + "_pw"], f8)
        cpb = np.asarray(inputs[pfx + "_pb"], f8)
        ckw, cvw = ckvw[:C], ckvw[C:]
        ckb_, cvb_ = ckvb[:C], ckvb[C:]
        cqw_e = CSC * cqw * sg[m][None, :]
        cqb_e = CSC * (cqb + cqw @ tsh[m])
        ckw_e = ckw * sg[mo][None, :]
        ckb_e = ckb_ + ckw @ tsh[mo]
        cvw_e = cvw * sg[mo][None, :]
        cvb_e = cvb_ + cvw @ tsh[mo]
        cpw_e = beta[m] * cpw
        pb_ca = beta[m] * cpb
        eff[f"cq_{m}"] = (cqw_e, cqb_e)
        eff[f"ck_{m}"] = (ckw_e, ckb_e)
        rep[f"ca_{m}_vwT"] = cvw_e.T.astype(bf)
        rep[f"ca_{m}_pwT"] = cpw_e.T.astype(bf)
        rep[f"ca_{m}_vb"] = cvb_e.reshape(C, 1).astype(bf)
        rep[f"pb_comb_{m}"] = (pb_sa + pb_ca).reshape(C, 1).astype(np.float32)

    for m, mo in (("r", "t"), ("t", "r")):
        rep[f"ca_from_{m}"] = np.concatenate(
            [eff[f"cq_{m}"][0].T, eff[f"ck_{mo}"][0].T], axis=1
        ).astype(bf)

    # per-core gram corrections from per-sample channel sums
    per_core = []
    for b in range(N_CORES):
        rowsum = {m: means[m][b] * HW for m in ("r", "t")}
        pc = {}
        for m, mo in (("r", "t"), ("t", "r")):
            cqw_e, cqb_e = eff[f"cq_{m}"]
            ckw_e, ckb_e = eff[f"ck_{m}"]
            r_q = cqw_e @ rowsum[m]
            r_k = ckw_e @ rowsum[mo]
            G = (np.outer(cqb_e, r_k) + np.outer(r_q, ckb_e)
                 + HW * np.outer(cqb_e, ckb_e))
            gex = np.empty((C, HD), np.float32)
            for n in range(NH):
                s = slice(n * HD, (n + 1) * HD)
                gex[s, :] = G[s, s]
            pc[f"gcorr_{m}"] = gex
        per_core.append(pc)
    return rep, per_core


# --------------------------------------------------------------------------
# Entry point
# --------------------------------------------------------------------------
_CACHE = {}


def _get(name, builder):
    if name not in _CACHE:
        _CACHE[name] = builder()
    return _CACHE[name]


def kernel(**inputs):
    rgb = np.ascontiguousarray(np.asarray(inputs["rgb"], np.float32))
    thermal = np.ascontiguousarray(np.asarray(inputs["thermal"], np.float32))
    cores = list(range(N_CORES))

    xr = rgb.reshape(B, C, HW)
    xt = thermal.reshape(B, C, HW)

    # ---- launch 1: stats ----
    nc_s = _get("stats", _build_stats)
    in_maps = [{"xr": xr[b], "xt": xt[b]} for b in range(N_CORES)]
    res_s = run_bass_kernel_spmd(nc_s, in_maps, core_ids=cores)
    core_stats = np.stack([res_s.results[b]["stats"] for b in range(N_CORES)])
    LAST_RUN_INFO["stats_exec_ns"] = res_s.exec_time_ns

    # ---- host folding ----
    rep, per_core = _fold(inputs, core_stats)

    # ---- launch 2: main ----
    nc_m = _get("main", _build_main)
    in_maps = []
    for b in range(N_CORES):
        im = {"xr": xr[b], "xt": xt[b]}
        im.update(rep)
        im.update(per_core[b])
        in_maps.append(im)
    res_m = run_bass_kernel_spmd(nc_m, in_maps, core_ids=cores)
    LAST_RUN_INFO["main_exec_ns"] = res_m.exec_time_ns
    LAST_RUN_INFO["main_mean_exec_ns"] = res_m.mean_exec_time_ns

    out = np.stack([res_m.results[b]["out"] for b in range(N_CORES)])
    return out.reshape(B, 2 * C, H, W)

